# revision 31
# baseline (speedup 1.0000x reference)
"""Trainium2 Bass kernel for fused multi-head causal attention.

Module: out = o_proj(causal_attention(rope_swapped(qkv_proj(x)))).
Shapes: x [2, 2048, 2048], 16 heads, head_dim 128.

Sharding (8 cores): batch (2) x head-group (4 groups of 4 heads).
Each core computes qkv projection + attention for its 4 heads of its
batch, then a partial o_proj against its slice of w_o rows.  The
all-reduce after o_proj is done host-side by summing the 4 partials
per batch (mathematically identical, avoids device collectives).

Key optimizations over the bf16 baseline:
 - QKV projection runs in fp8-e4m3 DoubleRow mode with a residual
   split: x ~ x8 + xr, w ~ w8 + wr (each fp8), and
   x@w ~ x8@w8 + x8@wr + xr@w8.  Each product contracts K=256 per
   DoubleRow matmul at 0.5 cycles/row, so the projection costs 0.75x
   the bf16 version with ~bf16 accuracy (the dropped xr@wr term is
   O(2^-8) relative).  w_qkv values are tiny (std 1/sqrt(D)), well
   inside e4m3's subnormal range, so w is pre-scaled by 64; the scale
   is folded into the rope tables (exactly) and the softmax
   reciprocal (exactly).
 - Softmax denominators come from matmuls with p as the *stationary*
   operand and a ones column as the moving operand (out [sq,128 x 1]),
   instead of ones-stationary (out [1 x sq]).  The PE costs
   out-free-size cycles per matmul, so each denominator column is ~1
   cycle instead of 512.  PV is likewise emitted as [sq, hd] tiles
   (p stationary) so normalization is a per-partition scalar multiply
   on the DVE; the normalized tiles are re-transposed on the PE
   (128 cycles per 128x128 tile) into the [hd, s] layout o_proj needs.
 - V projection uses x^T chunks as the stationary operand, producing
   v directly in [s, hd] layout: no separate transpose pass.
 - o_proj for s-chunk j is interleaved into the attention of chunk
   j+1 (deferred emission) so the PE stays busy while the Act engine
   grinds through the next chunk's exponentials.
 - The partial o_proj output is DMAed in bf16 and upcast/summed on the
   host (halves output traffic; adds noise far below bf16 matmul
   noise).
 - qkv bias matmuls are skipped when b_qkv is all zeros (it is, by
   construction of the reference); a generic bias path remains.
"""

import math

import ml_dtypes
import numpy as np

S = 2048
D = 2048
HD = 128
NH = 16
N_CORES = 8
SQ = 512          # free-dim chunk for matmuls / psum tiles
NJ = S // SQ      # 4 s-chunks
NKK = 8           # contraction pair-chunks (K=256 each) for fp8 DoubleRow
NST = S // 128    # 16 s-tiles of 128
WS = 64.0         # host-side scale applied to w_qkv before fp8 split
BF16 = ml_dtypes.bfloat16
F8 = ml_dtypes.float8_e4m3

_MODULE_CACHE = {}


def _build_module(use_bias):
    from contextlib import ExitStack

    import concourse.bass as bass
    import concourse.bacc as bacc
    import concourse.mybir as mybir
    import concourse.tile as tile

    f32 = mybir.dt.float32
    bf16 = mybir.dt.bfloat16
    f8 = mybir.dt.float8e4
    ts = bass.ts
    DR = mybir.MatmulPerfMode.DoubleRow
    MULT = mybir.AluOpType.mult

    nc = bacc.Bacc("TRN2", target_bir_lowering=False, debug=False,
                   num_devices=N_CORES)

    # DRAM I/O (identical program on all cores; per-core data differs)
    x8_d = nc.dram_tensor("x8", [D, S], f8, kind="ExternalInput").ap()
    xr_d = nc.dram_tensor("xr", [D, S], f8, kind="ExternalInput").ap()
    w8_d = nc.dram_tensor("w8", [D, 12 * 128], f8, kind="ExternalInput").ap()
    wr_d = nc.dram_tensor("wr", [D, 12 * 128], f8, kind="ExternalInput").ap()
    wo8_d = nc.dram_tensor("wo8", [2, 128, 2, D], f8, kind="ExternalInput").ap()
    wor_d = nc.dram_tensor("wor", [2, 128, 2, D], f8, kind="ExternalInput").ap()
    stab_d = nc.dram_tensor("stab", [128, S], bf16, kind="ExternalInput").ap()
    ctab_d = nc.dram_tensor("ctab", [128, S], bf16, kind="ExternalInput").ap()
    cmask_d = nc.dram_tensor("cmask", [128, 128], bf16, kind="ExternalInput").ap()
    ident_d = nc.dram_tensor("ident", [128, 128], bf16, kind="ExternalInput").ap()
    if use_bias:
        bq_d = nc.dram_tensor("bq", [1, 12 * 128], bf16,
                              kind="ExternalInput").ap()
    out_d = nc.dram_tensor("out", [S, D], bf16, kind="ExternalOutput").ap()

    inv_sqrt_hd = 1.0 / math.sqrt(HD)

    with tile.TileContext(nc) as tc, ExitStack() as ctx:
        # ---- long-lived SBUF pools -------------------------------------
        const_p = ctx.enter_context(tc.tile_pool(name="const", bufs=1))
        qk_p = ctx.enter_context(tc.tile_pool(name="qk", bufs=8))
        v_p = ctx.enter_context(tc.tile_pool(name="v", bufs=2))
        p_p = ctx.enter_context(tc.tile_pool(name="p", bufs=5))
        asq_p = ctx.enter_context(tc.tile_pool(name="asq", bufs=4))
        attnT_p = ctx.enter_context(tc.tile_pool(name="attnT", bufs=4))
        rc_p = ctx.enter_context(tc.tile_pool(name="rc", bufs=2))
        outsb_p = ctx.enter_context(tc.tile_pool(name="outsb", bufs=3))

        # ---- phase-A pools: x/w stay resident into phase B (deferred V
        # chunks read them); tables/scratch/psum close after phase A to
        # make room for the o_proj weights ----
        ctx2b = ctx.enter_context(ExitStack())
        xt_p = ctx2b.enter_context(tc.tile_pool(name="xt", bufs=2))
        w_p = ctx2b.enter_context(tc.tile_pool(name="w", bufs=3))
        ctx2 = ctx.enter_context(ExitStack())
        tab_p = ctx2.enter_context(tc.tile_pool(name="tab", bufs=2))
        scr_p = ctx2.enter_context(tc.tile_pool(name="scr", bufs=8))
        psA = ctx2.enter_context(
            tc.tile_pool(name="psA", bufs=8, space=bass.MemorySpace.PSUM))

        # ---- DMA in, ordered by first consumption ----------------------
        # One big DMA per tensor: the DMA cost is dominated by a fixed
        # per-descriptor overhead on the HWDGE ring, so batching matters.
        def load_w(dram, pair, name, nchunks):
            t = w_p.tile([128, 16, 768], f8, tag="w", name=name)
            step = 16 // nchunks
            for q in range(nchunks):
                nc.sync.dma_start(
                    out=t[:, q * step:(q + 1) * step, :],
                    in_=dram[q * step * 128:(q + 1) * step * 128,
                             pair * 768:(pair + 1) * 768].rearrange(
                        "(s p) c -> p s c", p=128))
            return t

        def load_x(dram, name):
            t = xt_p.tile([128, 16, S], f8, tag="xt", name=name)
            for kk in range(NKK):
                nc.sync.dma_start(
                    out=t[:, 2 * kk:2 * kk + 2, :],
                    in_=dram[256 * kk:256 * kk + 256, :].rearrange(
                        "(s p) c -> p s c", p=128))
            return t

        w8_t = [load_w(w8_d, 0, "w8p0", 4), None]
        x8_t = load_x(x8_d, "x8")
        xr_t = load_x(xr_d, "xr")
        wr_t = [load_w(wr_d, 0, "wrp0", 4), None]

        stab = tab_p.tile([128, S], bf16, tag="tab", name="stab")
        nc.sync.dma_start(out=stab[:], in_=stab_d[:])
        ctab = tab_p.tile([128, S], bf16, tag="tab", name="ctab")
        nc.sync.dma_start(out=ctab[:], in_=ctab_d[:])
        cmask = const_p.tile([128, 128], bf16, tag="c0", name="cmask")
        nc.sync.dma_start(out=cmask[:], in_=cmask_d[:])
        ident = const_p.tile([128, 128], bf16, tag="c1", name="ident")
        nc.sync.dma_start(out=ident[:], in_=ident_d[:])
        ones_col = const_p.tile([128, 1], bf16, tag="c2", name="ones_col")
        # denom stays unscaled: asq = apsum/denom keeps the V-side WS factor,
        # i.e. asq = WS*attn, placing the fp8 split of attn in normal range;
        # the host divides the output partials by WS*WS (attn and w_o scales)
        nc.vector.memset(ones_col[:], 1.0)
        if use_bias:
            bias_sb = const_p.tile([1, 12 * 128], bf16, tag="c3",
                                   name="bias_sb")
            nc.sync.dma_start(out=bias_sb[:], in_=bq_d[:])
            ones_row = const_p.tile([1, SQ], bf16, tag="c4", name="ones_row")
            nc.vector.memset(ones_row[:], 1.0)
            ones_m = const_p.tile([1, 128], bf16, tag="c5", name="ones_m")
            nc.vector.memset(ones_m[:], 1.0)

        w8_t[1] = load_w(w8_d, 1, "w8p1", 2)

        PRODS = ((0, 0), (0, 1), (1, 0))  # (w residual?, x residual?)

        def rope(j, accs, dq, dk):
            """accs = [q_lo, q_hi, k_lo, k_hi] pair-interleaved psum tiles
            (scaled by WS); writes per-head contiguous rotated [128, SQ]
            slices into dq[0]/dq[1]/dk[0]/dk[1] (tables carry the 1/WS):
              rot_lo = lo*sin - hi*cos ; rot_hi = hi*sin + lo*cos.
            All 8 psum-reading multiplies go first (DVE) so the psum banks
            free as early as possible; the sbuf-only combines run on the
            otherwise-idle GpSimd engine."""
            sl = stab[:, ts(j, SQ)]
            cl = ctab[:, ts(j, SQ)]
            # Act drains each psum acc to sbuf once (psum bank freed after a
            # single 0.6us copy), DVE multiplies from sbuf at 2x rate with
            # in-place reuse, Pool does the sbuf-only combines.
            cc = []
            for A in accs:
                c = scr_p.tile([128, SQ], bf16, tag="scr", name="c")
                nc.scalar.copy(c[:], A[:])
                cc.append(c)
            tt = []
            for cA, cB in ((cc[0], cc[1]), (cc[2], cc[3])):
                n1 = scr_p.tile([128, SQ], bf16, tag="scr", name="n1")
                nc.vector.tensor_mul(n1[:], cB[:], cl)   # B*cos
                nc.vector.tensor_mul(cB[:], cB[:], sl)   # B*sin (in place)
                n2 = scr_p.tile([128, SQ], bf16, tag="scr", name="n2")
                nc.vector.tensor_mul(n2[:], cA[:], cl)   # A*cos
                nc.vector.tensor_mul(cA[:], cA[:], sl)   # A*sin (in place)
                tt.append((cA, n1, cB, n2))
            for (t1, t2, t3, t4), dsts in zip(tt, (dq, dk)):
                for hh in range(2):
                    hs = slice(64 * hh, 64 * hh + 64)
                    nc.gpsimd.tensor_sub(dsts[hh][0:64, ts(j, SQ)],
                                         t1[hs, :], t2[hs, :])
                    nc.gpsimd.tensor_add(dsts[hh][64:128, ts(j, SQ)],
                                         t3[hs, :], t4[hs, :])

        # ---- phase A: QKV projection (fp8 DoubleRow, residual split) ---
        qT = [[None, None], [None, None]]
        kT = [[None, None], [None, None]]
        vs = [None, None]

        def qk_chunk(pair, j):
            accs = [psA.tile([128, SQ], f32, tag="ps", name="qkacc")
                    for _ in range(4)]
            for pi, (wres, xres) in enumerate(PRODS):
                wt = (wr_t if wres else w8_t)[pair]
                xt = (xr_t if xres else x8_t)
                for kk in range(NKK):
                    for m in range(4):
                        nc.tensor.matmul(
                            accs[m][:],
                            wt[:, 2 * kk:2 * kk + 2, ts(m, 128)],
                            xt[:, 2 * kk:2 * kk + 2, ts(j, SQ)],
                            start=(pi == 0 and kk == 0),
                            stop=(pi == 2 and kk == NKK - 1 and not use_bias),
                            perf_mode=DR)
            if use_bias:
                gm = pair * 6
                for m in range(4):
                    nc.tensor.matmul(
                        accs[m][:],
                        bias_sb[0:1, (gm + m) * 128:(gm + m + 1) * 128],
                        ones_row[0:1, :],
                        start=False, stop=True)
            rope(j, accs, qT[pair], kT[pair])

        def v_chunk(pair, st, pool=None, dve_copy=False):
            vp = (pool or psA).tile([128, 256], f32, tag="ps", name="vacc")
            for pi, (wres, xres) in enumerate(PRODS):
                wt = (wr_t if wres else w8_t)[pair]
                xt = (xr_t if xres else x8_t)
                for kk in range(NKK):
                    nc.tensor.matmul(
                        vp[:],
                        xt[:, 2 * kk:2 * kk + 2, ts(st, 128)],
                        wt[:, 2 * kk:2 * kk + 2, 512:768],
                        start=(pi == 0 and kk == 0),
                        stop=(pi == 2 and kk == NKK - 1 and not use_bias),
                        perf_mode=DR)
            if use_bias:
                nc.tensor.matmul(
                    vp[:], ones_m[0:1, :],
                    bias_sb[0:1, pair * 768 + 512:pair * 768 + 768],
                    start=False, stop=True)
            if dve_copy:
                nc.vector.tensor_copy(vs[pair][:, st, :], vp[:])
            else:
                nc.scalar.copy(vs[pair][:, st, :], vp[:])

        for pair in range(2):
            if pair == 1:
                # reuses w8p0's pool slot, freed by pair-0's last matmul
                wr_t[1] = load_w(wr_d, 1, "wrp1", 2)
            for hh in range(2):
                qT[pair][hh] = qk_p.tile([128, S], bf16, tag="qk", name="qT")
                kT[pair][hh] = qk_p.tile([128, S], bf16, tag="qk", name="kT")
            vs[pair] = v_p.tile([128, NST, 256], bf16, tag="v", name="vs")
            # interleave V chunks between Q/K chunks to spread the psum
            # pressure (V releases its psum to the Act engine quickly,
            # Q/K releases it to the slower DVE rope).
            if pair == 0:
                qk_chunk(0, 0)
                qk_chunk(0, 1)
                for st in range(0, 8):
                    v_chunk(0, st)
                qk_chunk(0, 2)
                qk_chunk(0, 3)
                for st in range(8, NST):
                    v_chunk(0, st)
            else:
                # V last: it has no rope dependency, so the PE keeps busy
                # while the DVE/Pool rope backlog drains, and the phase-A
                # psum pool (which phase B's lg pool reuses) frees early
                qk_chunk(1, 0)
                qk_chunk(1, 1)
                qk_chunk(1, 2)
                qk_chunk(1, 3)
                for st in range(0, 4):
                    v_chunk(1, st)

        ctx2.close()   # release tab/scr + phase-A psum; x/w stay for
        # the deferred V chunks woven into phase B

        wo_p = ctx.enter_context(tc.tile_pool(name="wo", bufs=4))
        wo8_sb, wor_sb = [], []
        for hp in range(2):
            t = wo_p.tile([128, 2, D], f8, tag="wo", name="wo8")
            nc.sync.dma_start(out=t[:], in_=wo8_d[hp])
            wo8_sb.append(t)
        for hp in range(2):
            t = wo_p.tile([128, 2, D], f8, tag="wo", name="wor")
            nc.sync.dma_start(out=t[:], in_=wor_d[hp])
            wor_sb.append(t)

        # attn (x WS) split into fp8 value+residual, in DoubleRow head-pair
        # layout [hd, head-in-pair, s] for the fp8 o_proj
        attnT8 = [attnT_p.tile([128, 2, S], f8, tag="attnT", name="aT8")
                  for _ in range(2)]
        attnTr = [attnT_p.tile([128, 2, S], f8, tag="attnT", name="aTr")
                  for _ in range(2)]

        # ---- phase B psum pools ----------------------------------------
        lg_p = ctx.enter_context(
            tc.tile_pool(name="lg", bufs=3, space=bass.MemorySpace.PSUM))
        dfr_p = ctx.enter_context(
            tc.tile_pool(name="dfr", bufs=2, space=bass.MemorySpace.PSUM))
        acc_p = ctx.enter_context(
            tc.tile_pool(name="acc", bufs=2, space=bass.MemorySpace.PSUM))
        dps_p = ctx.enter_context(
            tc.tile_pool(name="dps", bufs=1, space=bass.MemorySpace.PSUM))

        deferred = []   # (tag, fn): PE work drip-fed into later loops

        def pop_deferred(n):
            for _ in range(n):
                if deferred:
                    deferred.pop(0)[1]()

        def drain_v(pair, max_st):
            """Force any still-queued V chunks the coming batch reads."""
            for item in [it for it in deferred
                         if it[0] is not None and it[0][0] == pair
                         and it[0][1] <= max_st]:
                deferred.remove(item)
                item[1]()

        def transp_thunk(j, pair, hh, s, asq):
            t = dfr_p.tile([128, 1024], bf16, tag="ps", name="pt")
            nc.tensor.transpose(t[:, 0:128], asq[hh][:, s, :], ident[:])
            cols = slice(j * SQ + s * 128, j * SQ + (s + 1) * 128)
            with nc.allow_low_precision(reason="fp8 value+residual split"):
                nc.scalar.copy(attnT8[pair][:, hh, cols], t[:, 0:128])
                nc.vector.tensor_sub(attnTr[pair][:, hh, cols],
                                     t[:, 0:128], attnT8[pair][:, hh, cols])

        def oproj_thunk(st, egp):
            ot = outsb_p.tile([128, 2 * SQ], bf16, tag="outsb", name="ot")
            for ei in range(2):
                eg = 2 * egp + ei
                op = dfr_p.tile([128, SQ], f32, tag="ps", name="oproj")
                prods = ((attnT8, wo8_sb), (attnT8, wor_sb), (attnTr, wo8_sb))
                for pi, (at, wt) in enumerate(prods):
                    for hp in range(2):
                        nc.tensor.matmul(
                            op[:],
                            at[hp][:, :, ts(st, 128)],
                            wt[hp][:, :, ts(eg, SQ)],
                            start=(pi == 0 and hp == 0),
                            stop=(pi == 2 and hp == 1),
                            perf_mode=DR)
                nc.vector.tensor_copy(ot[:, ts(ei, SQ)], op[:])
            nc.sync.dma_start(
                out=out_d[st * 128:(st + 1) * 128,
                          2 * egp * SQ:2 * (egp + 1) * SQ],
                in_=ot[:])

        # late V chunks: woven into phase B as PE filler for the
        # Act-latency-bound early attention batches
        for st in range(4, NST):
            deferred.append(
                ((1, st), lambda st=st:
                 v_chunk(1, st, pool=dfr_p, dve_copy=True)))

        # ---- phase B: attention (per s-chunk, both head pairs) ---------
        pending_fin = [None]

        for j in range(NJ):
            ndiag = 4 * j + 4
            for pair in range(2):
                drain_v(pair, 4 * j + 3)
                apsum = [acc_p.tile([128, 4, 128], f32, tag="acc",
                                    name="apsum") for _ in range(2)]
                dpsum = dps_p.tile([128, SQ], f32, tag="dps", name="dpsum")

                def emit_pv(i, r, pts, _ap=apsum, _dp=dpsum, _pair=pair,
                            _nd=ndiag):
                    first = (i == 0)
                    last = (i == _nd - 1)
                    s0 = r if r > 0 else 0
                    for hh in range(2):
                        for s in range(s0, 4):
                            nc.tensor.matmul(
                                _ap[hh][:, s, :],
                                pts[hh][:, ts(s, 128)],
                                vs[_pair][:, i, ts(hh, 128)],
                                start=(first and s == s0),
                                stop=(last and s == 3),
                                skip_group_check=True)
                            c = hh * 4 + s
                            nc.tensor.matmul(
                                _dp[:, c:c + 1],
                                pts[hh][:, ts(s, 128)],
                                ones_col[:],
                                start=(first and hh == 0 and s == s0),
                                stop=(last and hh == 1 and s == 3),
                                skip_group_check=True)

                prev = None   # (i, r, p_tiles)
                for i in range(ndiag):
                    r = i - 4 * j   # >=0 on diagonal tiles
                    off = 128 * r if r > 0 else 0
                    lg = []
                    for hh in range(2):
                        t = lg_p.tile([128, SQ], f32, tag="ps", name="lg")
                        nc.tensor.matmul(
                            t[:, off:SQ],
                            kT[pair][hh][:, ts(i, 128)],
                            qT[pair][hh][:, j * SQ + off:(j + 1) * SQ],
                            start=True, stop=(r < 0))
                        if r >= 0:
                            # causal mask folded into the psum group: adds
                            # the [-9e15] triangle via ident.T @ cmask
                            nc.tensor.matmul(
                                t[:, off:off + 128], ident[:], cmask[:],
                                start=False, stop=True,
                                skip_group_check=True)
                        lg.append(t)
                    pts = []
                    for hh in range(2):
                        p_t = p_p.tile([128, SQ], bf16, tag="p", name="p_t")
                        nc.scalar.activation(
                            p_t[:, off:SQ], lg[hh][:, off:SQ],
                            mybir.ActivationFunctionType.Exp,
                            scale=inv_sqrt_hd)
                        pts.append(p_t)
                    if i == 0 and pending_fin[0] is not None:
                        pending_fin[0]()
                        pending_fin[0] = None
                    if prev is not None:
                        emit_pv(*prev)
                    if i >= 1:
                        pop_deferred((2 if j < 3 else 3) if r >= 0 else 1)
                    prev = (i, r, pts)
                emit_pv(*prev)

                def fin(_ap=apsum, _dp=dpsum, _j=j, _pair=pair):
                    # 1/(WS * denom); normalize each [sq, hd] slice; queue
                    # the re-transposes (and o_proj once the chunk is done)
                    rc = rc_p.tile([128, 8], f32, tag="rc", name="rc")
                    nc.vector.reciprocal(rc[:], _dp[:, 0:8])
                    asq = [asq_p.tile([128, 4, 128], bf16, tag="asq",
                                      name="asq") for _ in range(2)]
                    for hh in range(2):
                        nc.vector.tensor_mul(
                            asq[hh][:, :, :], _ap[hh][:, :, :],
                            rc[:, 4 * hh:4 * hh + 4].unsqueeze(2)
                            .broadcast_to([128, 4, 128]))
                    for hh in range(2):
                        for s in range(4):
                            deferred.append(
                                (None, lambda hh=hh, s=s, asq=asq:
                                 transp_thunk(_j, _pair, hh, s, asq)))
                    if _pair == 1:
                        for st in range(4 * _j, 4 * _j + 4):
                            for egp in range(2):
                                deferred.append(
                                    (None, lambda st=st, egp=egp:
                                     oproj_thunk(st, egp)))
                pending_fin[0] = fin
        pending_fin[0]()
        pop_deferred(len(deferred))

    nc.compile()
    return nc


def _host_inputs(x, w_qkv, b_qkv, w_o):
    """Build the 8 per-core input maps."""
    x = np.asarray(x, dtype=np.float32)
    w_qkv = np.asarray(w_qkv, dtype=np.float32)
    b_qkv = np.asarray(b_qkv, dtype=np.float32)
    w_o = np.asarray(w_o, dtype=np.float32)
    use_bias = bool(np.any(b_qkv != 0.0))

    # rope tables (reference swaps sin/cos roles; we follow the math:
    # q_rot = q*sin(emb) + rotate_half(q)*cos(emb)).  Tables are divided
    # by WS to cancel the fp8 weight scaling.
    inv_freq = 1.0 / (10000.0 ** (np.arange(0, HD, 2, dtype=np.float32) / HD))
    t = np.arange(S, dtype=np.float32)
    freq = np.einsum("s,f->sf", t, inv_freq)          # [S, 64]
    sinT = (np.sin(freq).T / WS).astype(np.float32)   # [64, S]
    cosT = (np.cos(freq).T / WS).astype(np.float32)
    stab = np.concatenate([sinT, sinT], 0).astype(BF16)   # [128, S]
    ctab = np.concatenate([cosT, cosT], 0).astype(BF16)

    p_idx = np.arange(128)[:, None]
    f_idx = np.arange(128)[None, :]
    cmask = np.where(f_idx >= p_idx, 0.0, -9e15).astype(BF16)
    ident = np.eye(128, dtype=np.float32).astype(BF16)

    def head_w(h):
        base = h * 3 * HD
        return (w_qkv[:, base:base + HD],
                w_qkv[:, base + HD:base + 2 * HD],
                w_qkv[:, base + 2 * HD:base + 3 * HD])

    def head_b(h):
        base = h * 3 * HD
        return (b_qkv[base:base + HD],
                b_qkv[base + HD:base + 2 * HD],
                b_qkv[base + 2 * HD:base + 3 * HD])

    def fp8split(a):
        a8 = a.astype(F8)
        ar = (a - a8.astype(np.float32)).astype(F8)
        return a8, ar

    in_maps = []
    for c in range(N_CORES):
        b = c // 4
        heads = [4 * (c % 4) + i for i in range(4)]
        x8, xr = fp8split(np.ascontiguousarray(x[b].T))

        mats, bvec = [], []
        for pair in range(2):
            ha, hb = heads[2 * pair], heads[2 * pair + 1]
            wq_a, wk_a, wv_a = head_w(ha)
            wq_b, wk_b, wv_b = head_w(hb)
            bq_a, bk_a, bv_a = head_b(ha)
            bq_b, bk_b, bv_b = head_b(hb)
            mats += [
                np.concatenate([wq_a[:, :64], wq_b[:, :64]], 1),
                np.concatenate([wq_a[:, 64:], wq_b[:, 64:]], 1),
                np.concatenate([wk_a[:, :64], wk_b[:, :64]], 1),
                np.concatenate([wk_a[:, 64:], wk_b[:, 64:]], 1),
                wv_a, wv_b,
            ]
            bvec += [
                np.concatenate([bq_a[:64], bq_b[:64]]),
                np.concatenate([bq_a[64:], bq_b[64:]]),
                np.concatenate([bk_a[:64], bk_b[:64]]),
                np.concatenate([bk_a[64:], bk_b[64:]]),
                bv_a, bv_b,
            ]
        wq_all = np.concatenate(mats, 1) * WS                  # [D, 1536]
        w8, wr = fp8split(wq_all.astype(np.float32))
        wo_all = np.concatenate(
            [w_o[h * HD:(h + 1) * HD, :] for h in heads], 0) * WS
        # [4*128, D] -> [hp, p, ki, D] DoubleRow head-pair layout
        wo_hp = np.ascontiguousarray(
            wo_all.reshape(2, 2, HD, D).transpose(0, 2, 1, 3))
        wo8, wor = fp8split(wo_hp.astype(np.float32))

        im = {
            "x8": x8, "xr": xr, "w8": w8, "wr": wr,
            "wo8": wo8, "wor": wor,
            "stab": stab, "ctab": ctab, "cmask": cmask, "ident": ident,
        }
        if use_bias:
            im["bq"] = (np.concatenate(bvec)[None, :] * WS).astype(BF16)
        in_maps.append(im)
    return in_maps, use_bias


def _run(in_maps, use_bias, trace=False):
    from concourse.bass_utils import run_bass_kernel_spmd
    key = ("nc", use_bias)
    if key not in _MODULE_CACHE:
        _MODULE_CACHE[key] = _build_module(use_bias)
        _MODULE_CACHE["nc"] = _MODULE_CACHE[key]
    nc = _MODULE_CACHE[key]
    return nc, run_bass_kernel_spmd(nc, in_maps, core_ids=list(range(N_CORES)),
                                    trace=trace)


def kernel(x, w_qkv, b_qkv, w_o, b_o, _trace=False, _return_res=False):
    in_maps, use_bias = _host_inputs(x, w_qkv, b_qkv, w_o)
    nc, res = _run(in_maps, use_bias, trace=_trace)
    out = np.zeros((2, S, D), dtype=np.float32)
    for c in range(N_CORES):
        out[c // 4] += np.asarray(res.results[c]["out"], dtype=np.float32)
    out *= 1.0 / (WS * WS)   # undo the attn and w_o fp8 scaling
    out += np.asarray(b_o, dtype=np.float32)[None, None, :]
    if _return_res:
        return out, res
    return out


# revision 32
# speedup vs baseline: 1.0555x; 1.0555x over previous
"""Trainium2 Bass kernel for fused multi-head causal attention.

Module: out = o_proj(causal_attention(rope_swapped(qkv_proj(x)))).
Shapes: x [2, 2048, 2048], 16 heads, head_dim 128.

Sharding (8 cores): batch (2) x head-group (4 groups of 4 heads).
Each core computes qkv projection + attention for its 4 heads of its
batch, then a partial o_proj against its slice of w_o rows.  The
all-reduce after o_proj is done host-side by summing the 4 partials
per batch (mathematically identical, avoids device collectives).

Key optimizations over the bf16 baseline:
 - QKV projection runs in fp8-e4m3 DoubleRow mode with a residual
   split: x ~ x8 + xr, w ~ w8 + wr (each fp8), and
   x@w ~ x8@w8 + x8@wr + xr@w8.  Each product contracts K=256 per
   DoubleRow matmul at 0.5 cycles/row, so the projection costs 0.75x
   the bf16 version with ~bf16 accuracy (the dropped xr@wr term is
   O(2^-8) relative).  w_qkv values are tiny (std 1/sqrt(D)), well
   inside e4m3's subnormal range, so w is pre-scaled by 64; the scale
   is folded into the rope tables (exactly) and the softmax
   reciprocal (exactly).
 - Softmax denominators come from matmuls with p as the *stationary*
   operand and a ones column as the moving operand (out [sq,128 x 1]),
   instead of ones-stationary (out [1 x sq]).  The PE costs
   out-free-size cycles per matmul, so each denominator column is ~1
   cycle instead of 512.  PV is likewise emitted as [sq, hd] tiles
   (p stationary) so normalization is a per-partition scalar multiply
   on the DVE; the normalized tiles are re-transposed on the PE
   (128 cycles per 128x128 tile) into the [hd, s] layout o_proj needs.
 - V projection uses x^T chunks as the stationary operand, producing
   v directly in [s, hd] layout: no separate transpose pass.
 - o_proj for s-chunk j is interleaved into the attention of chunk
   j+1 (deferred emission) so the PE stays busy while the Act engine
   grinds through the next chunk's exponentials.
 - The partial o_proj output is DMAed in bf16 and upcast/summed on the
   host (halves output traffic; adds noise far below bf16 matmul
   noise).
 - qkv bias matmuls are skipped when b_qkv is all zeros (it is, by
   construction of the reference); a generic bias path remains.
"""

import math

import ml_dtypes
import numpy as np

S = 2048
D = 2048
HD = 128
NH = 16
N_CORES = 8
SQ = 512          # free-dim chunk for matmuls / psum tiles
NJ = S // SQ      # 4 s-chunks
NKK = 8           # contraction pair-chunks (K=256 each) for fp8 DoubleRow
NST = S // 128    # 16 s-tiles of 128
WS = 64.0         # host-side scale applied to w_qkv before fp8 split
BF16 = ml_dtypes.bfloat16
F8 = ml_dtypes.float8_e4m3

_MODULE_CACHE = {}


def _build_module(use_bias):
    from contextlib import ExitStack

    import concourse.bass as bass
    import concourse.bacc as bacc
    import concourse.mybir as mybir
    import concourse.tile as tile

    f32 = mybir.dt.float32
    bf16 = mybir.dt.bfloat16
    f8 = mybir.dt.float8e4
    ts = bass.ts
    DR = mybir.MatmulPerfMode.DoubleRow
    MULT = mybir.AluOpType.mult

    nc = bacc.Bacc("TRN2", target_bir_lowering=False, debug=False,
                   num_devices=N_CORES)

    # DRAM I/O (identical program on all cores; per-core data differs)
    x8_d = nc.dram_tensor("x8", [D, S], f8, kind="ExternalInput").ap()
    xr_d = nc.dram_tensor("xr", [D, S], f8, kind="ExternalInput").ap()
    w8_d = nc.dram_tensor("w8", [D, 12 * 128], f8, kind="ExternalInput").ap()
    wr_d = nc.dram_tensor("wr", [D, 12 * 128], f8, kind="ExternalInput").ap()
    wo8_d = nc.dram_tensor("wo8", [2, 128, 2, D], f8, kind="ExternalInput").ap()
    wor_d = nc.dram_tensor("wor", [2, 128, 2, D], f8, kind="ExternalInput").ap()
    stab_d = nc.dram_tensor("stab", [128, S], bf16, kind="ExternalInput").ap()
    ctab_d = nc.dram_tensor("ctab", [128, S], bf16, kind="ExternalInput").ap()
    cmask_d = nc.dram_tensor("cmask", [128, 128], bf16, kind="ExternalInput").ap()
    ident_d = nc.dram_tensor("ident", [128, 128], bf16, kind="ExternalInput").ap()
    if use_bias:
        bq_d = nc.dram_tensor("bq", [1, 12 * 128], bf16,
                              kind="ExternalInput").ap()
    out_d = nc.dram_tensor("out", [S, D], bf16, kind="ExternalOutput").ap()

    inv_sqrt_hd = 1.0 / math.sqrt(HD)

    with tile.TileContext(nc) as tc, ExitStack() as ctx:
        # ---- long-lived SBUF pools -------------------------------------
        const_p = ctx.enter_context(tc.tile_pool(name="const", bufs=1))
        qk_p = ctx.enter_context(tc.tile_pool(name="qk", bufs=8))
        v_p = ctx.enter_context(tc.tile_pool(name="v", bufs=2))
        p_p = ctx.enter_context(tc.tile_pool(name="p", bufs=5))
        asq_p = ctx.enter_context(tc.tile_pool(name="asq", bufs=4))
        attnT_p = ctx.enter_context(tc.tile_pool(name="attnT", bufs=4))
        rc_p = ctx.enter_context(tc.tile_pool(name="rc", bufs=2))
        outsb_p = ctx.enter_context(tc.tile_pool(name="outsb", bufs=3))

        # ---- phase-A pools: x/w stay resident into phase B (deferred V
        # chunks read them); tables/scratch/psum close after phase A to
        # make room for the o_proj weights ----
        ctx2b = ctx.enter_context(ExitStack())
        xt_p = ctx2b.enter_context(tc.tile_pool(name="xt", bufs=2))
        w_p = ctx2b.enter_context(tc.tile_pool(name="w", bufs=3))
        ctx2 = ctx.enter_context(ExitStack())
        tab_p = ctx2.enter_context(tc.tile_pool(name="tab", bufs=2))
        scr_p = ctx2.enter_context(tc.tile_pool(name="scr", bufs=8))
        psA = ctx2.enter_context(
            tc.tile_pool(name="psA", bufs=8, space=bass.MemorySpace.PSUM))

        # ---- DMA in, ordered by first consumption ----------------------
        # One big DMA per tensor: the DMA cost is dominated by a fixed
        # per-descriptor overhead on the HWDGE ring, so batching matters.
        def load_w(dram, pair, name, nchunks):
            t = w_p.tile([128, 16, 768], f8, tag="w", name=name)
            step = 16 // nchunks
            for q in range(nchunks):
                nc.sync.dma_start(
                    out=t[:, q * step:(q + 1) * step, :],
                    in_=dram[q * step * 128:(q + 1) * step * 128,
                             pair * 768:(pair + 1) * 768].rearrange(
                        "(s p) c -> p s c", p=128))
            return t

        def load_x(dram, name):
            t = xt_p.tile([128, 16, S], f8, tag="xt", name=name)
            for kk in range(NKK):
                nc.sync.dma_start(
                    out=t[:, 2 * kk:2 * kk + 2, :],
                    in_=dram[256 * kk:256 * kk + 256, :].rearrange(
                        "(s p) c -> p s c", p=128))
            return t

        w8_t = [load_w(w8_d, 0, "w8p0", 4), None]
        x8_t = load_x(x8_d, "x8")
        xr_t = load_x(xr_d, "xr")
        wr_t = [load_w(wr_d, 0, "wrp0", 4), None]

        stab = tab_p.tile([128, S], bf16, tag="tab", name="stab")
        nc.sync.dma_start(out=stab[:], in_=stab_d[:])
        ctab = tab_p.tile([128, S], bf16, tag="tab", name="ctab")
        nc.sync.dma_start(out=ctab[:], in_=ctab_d[:])
        cmask = const_p.tile([128, 128], bf16, tag="c0", name="cmask")
        nc.sync.dma_start(out=cmask[:], in_=cmask_d[:])
        ident = const_p.tile([128, 128], bf16, tag="c1", name="ident")
        nc.sync.dma_start(out=ident[:], in_=ident_d[:])
        ones_col = const_p.tile([128, 1], bf16, tag="c2", name="ones_col")
        # denom stays unscaled: asq = apsum/denom keeps the V-side WS factor,
        # i.e. asq = WS*attn, placing the fp8 split of attn in normal range;
        # the host divides the output partials by WS*WS (attn and w_o scales)
        nc.vector.memset(ones_col[:], 1.0)
        if use_bias:
            bias_sb = const_p.tile([1, 12 * 128], bf16, tag="c3",
                                   name="bias_sb")
            nc.sync.dma_start(out=bias_sb[:], in_=bq_d[:])
            ones_row = const_p.tile([1, SQ], bf16, tag="c4", name="ones_row")
            nc.vector.memset(ones_row[:], 1.0)
            ones_m = const_p.tile([1, 128], bf16, tag="c5", name="ones_m")
            nc.vector.memset(ones_m[:], 1.0)

        w8_t[1] = load_w(w8_d, 1, "w8p1", 2)

        PRODS = ((0, 0), (0, 1), (1, 0))  # (w residual?, x residual?)

        def rope(j, accs, dq, dk):
            """accs = [q_lo, q_hi, k_lo, k_hi] pair-interleaved psum tiles
            (scaled by WS); writes per-head contiguous rotated [128, SQ]
            slices into dq[0]/dq[1]/dk[0]/dk[1] (tables carry the 1/WS):
              rot_lo = lo*sin - hi*cos ; rot_hi = hi*sin + lo*cos.
            All 8 psum-reading multiplies go first (DVE) so the psum banks
            free as early as possible; the sbuf-only combines run on the
            otherwise-idle GpSimd engine."""
            sl = stab[:, ts(j, SQ)]
            cl = ctab[:, ts(j, SQ)]
            # Act drains each psum acc to sbuf once (psum bank freed after a
            # single 0.6us copy), DVE multiplies from sbuf at 2x rate with
            # in-place reuse, Pool does the sbuf-only combines.
            cc = []
            for A in accs:
                c = scr_p.tile([128, SQ], bf16, tag="scr", name="c")
                nc.scalar.copy(c[:], A[:])
                cc.append(c)
            tt = []
            for cA, cB in ((cc[0], cc[1]), (cc[2], cc[3])):
                n1 = scr_p.tile([128, SQ], bf16, tag="scr", name="n1")
                nc.vector.tensor_mul(n1[:], cB[:], cl)   # B*cos
                nc.vector.tensor_mul(cB[:], cB[:], sl)   # B*sin (in place)
                n2 = scr_p.tile([128, SQ], bf16, tag="scr", name="n2")
                nc.vector.tensor_mul(n2[:], cA[:], cl)   # A*cos
                nc.vector.tensor_mul(cA[:], cA[:], sl)   # A*sin (in place)
                tt.append((cA, n1, cB, n2))
            for (t1, t2, t3, t4), dsts in zip(tt, (dq, dk)):
                for hh in range(2):
                    hs = slice(64 * hh, 64 * hh + 64)
                    nc.gpsimd.tensor_sub(dsts[hh][0:64, ts(j, SQ)],
                                         t1[hs, :], t2[hs, :])
                    nc.gpsimd.tensor_add(dsts[hh][64:128, ts(j, SQ)],
                                         t3[hs, :], t4[hs, :])

        # ---- phase A: QKV projection (fp8 DoubleRow, residual split) ---
        qT = [[None, None], [None, None]]
        kT = [[None, None], [None, None]]
        vs = [None, None]

        def qk_chunk(pair, j):
            accs = [psA.tile([128, SQ], f32, tag="ps", name="qkacc")
                    for _ in range(4)]
            for pi, (wres, xres) in enumerate(PRODS):
                wt = (wr_t if wres else w8_t)[pair]
                xt = (xr_t if xres else x8_t)
                for kk in range(NKK):
                    for m in range(4):
                        nc.tensor.matmul(
                            accs[m][:],
                            wt[:, 2 * kk:2 * kk + 2, ts(m, 128)],
                            xt[:, 2 * kk:2 * kk + 2, ts(j, SQ)],
                            start=(pi == 0 and kk == 0),
                            stop=(pi == 2 and kk == NKK - 1 and not use_bias),
                            perf_mode=DR)
            if use_bias:
                gm = pair * 6
                for m in range(4):
                    nc.tensor.matmul(
                        accs[m][:],
                        bias_sb[0:1, (gm + m) * 128:(gm + m + 1) * 128],
                        ones_row[0:1, :],
                        start=False, stop=True)
            rope(j, accs, qT[pair], kT[pair])

        def v_chunk(pair, st, pool=None, dve_copy=False):
            vp = (pool or psA).tile([128, 256], f32, tag="ps", name="vacc")
            for pi, (wres, xres) in enumerate(PRODS):
                wt = (wr_t if wres else w8_t)[pair]
                xt = (xr_t if xres else x8_t)
                for kk in range(NKK):
                    nc.tensor.matmul(
                        vp[:],
                        xt[:, 2 * kk:2 * kk + 2, ts(st, 128)],
                        wt[:, 2 * kk:2 * kk + 2, 512:768],
                        start=(pi == 0 and kk == 0),
                        stop=(pi == 2 and kk == NKK - 1 and not use_bias),
                        perf_mode=DR)
            if use_bias:
                nc.tensor.matmul(
                    vp[:], ones_m[0:1, :],
                    bias_sb[0:1, pair * 768 + 512:pair * 768 + 768],
                    start=False, stop=True)
            if dve_copy:
                nc.vector.tensor_copy(vs[pair][:, st, :], vp[:])
            else:
                nc.scalar.copy(vs[pair][:, st, :], vp[:])

        for pair in range(2):
            if pair == 1:
                # reuses w8p0's pool slot, freed by pair-0's last matmul
                wr_t[1] = load_w(wr_d, 1, "wrp1", 2)
            for hh in range(2):
                qT[pair][hh] = qk_p.tile([128, S], bf16, tag="qk", name="qT")
                kT[pair][hh] = qk_p.tile([128, S], bf16, tag="qk", name="kT")
            vs[pair] = v_p.tile([128, NST, 256], bf16, tag="v", name="vs")
            # interleave V chunks between Q/K chunks to spread the psum
            # pressure (V releases its psum to the Act engine quickly,
            # Q/K releases it to the slower DVE rope).
            if pair == 0:
                qk_chunk(0, 0)
                qk_chunk(0, 1)
                for st in range(0, 8):
                    v_chunk(0, st)
                qk_chunk(0, 2)
                qk_chunk(0, 3)
                for st in range(8, NST):
                    v_chunk(0, st)
            else:
                # V last: it has no rope dependency, so the PE keeps busy
                # while the DVE/Pool rope backlog drains, and the phase-A
                # psum pool (which phase B's lg pool reuses) frees early
                qk_chunk(1, 0)
                qk_chunk(1, 1)
                qk_chunk(1, 2)
                qk_chunk(1, 3)
                for st in range(0, 4):
                    v_chunk(1, st)

        ctx2.close()   # release tab/scr + phase-A psum; x/w stay for
        # the deferred V chunks woven into phase B

        wo_p = ctx.enter_context(tc.tile_pool(name="wo", bufs=4))
        wo8_sb, wor_sb = [], []
        for hp in range(2):
            t = wo_p.tile([128, 2, D], f8, tag="wo", name="wo8")
            nc.sync.dma_start(out=t[:], in_=wo8_d[hp])
            wo8_sb.append(t)
        for hp in range(2):
            t = wo_p.tile([128, 2, D], f8, tag="wo", name="wor")
            nc.sync.dma_start(out=t[:], in_=wor_d[hp])
            wor_sb.append(t)

        # attn (x WS) split into fp8 value+residual, in DoubleRow head-pair
        # layout [hd, head-in-pair, s] for the fp8 o_proj
        attnT8 = [attnT_p.tile([128, 2, S], f8, tag="attnT", name="aT8")
                  for _ in range(2)]
        attnTr = [attnT_p.tile([128, 2, S], f8, tag="attnT", name="aTr")
                  for _ in range(2)]

        # ---- phase B psum pools ----------------------------------------
        lg_p = ctx.enter_context(
            tc.tile_pool(name="lg", bufs=3, space=bass.MemorySpace.PSUM))
        dfr_p = ctx.enter_context(
            tc.tile_pool(name="dfr", bufs=2, space=bass.MemorySpace.PSUM))
        acc_p = ctx.enter_context(
            tc.tile_pool(name="acc", bufs=2, space=bass.MemorySpace.PSUM))
        dps_p = ctx.enter_context(
            tc.tile_pool(name="dps", bufs=1, space=bass.MemorySpace.PSUM))

        deferred = []   # (tag, fn): PE work drip-fed into later loops

        def pop_deferred(n):
            for _ in range(n):
                if deferred:
                    deferred.pop(0)[1]()

        def drain_v(pair, max_st):
            """Force any still-queued V chunks the coming batch reads."""
            for item in [it for it in deferred
                         if it[0] is not None and it[0][0] == pair
                         and it[0][1] <= max_st]:
                deferred.remove(item)
                item[1]()

        def transp_thunk(j, pair, hh, s, asq):
            t = dfr_p.tile([128, 1024], bf16, tag="ps", name="pt")
            nc.tensor.transpose(t[:, 0:128], asq[hh][:, s, :], ident[:])
            cols = slice(j * SQ + s * 128, j * SQ + (s + 1) * 128)
            with nc.allow_low_precision(reason="fp8 value+residual split"):
                nc.vector.tensor_copy(attnT8[pair][:, hh, cols], t[:, 0:128])
                nc.vector.tensor_sub(attnTr[pair][:, hh, cols],
                                     t[:, 0:128], attnT8[pair][:, hh, cols])

        def oproj_thunk(st, egp):
            ot = outsb_p.tile([128, 2 * SQ], bf16, tag="outsb", name="ot")
            for ei in range(2):
                eg = 2 * egp + ei
                op = dfr_p.tile([128, SQ], f32, tag="ps", name="oproj")
                prods = ((attnT8, wo8_sb), (attnT8, wor_sb), (attnTr, wo8_sb))
                for pi, (at, wt) in enumerate(prods):
                    for hp in range(2):
                        nc.tensor.matmul(
                            op[:],
                            at[hp][:, :, ts(st, 128)],
                            wt[hp][:, :, ts(eg, SQ)],
                            start=(pi == 0 and hp == 0),
                            stop=(pi == 2 and hp == 1),
                            perf_mode=DR)
                nc.vector.tensor_copy(ot[:, ts(ei, SQ)], op[:])
            nc.sync.dma_start(
                out=out_d[st * 128:(st + 1) * 128,
                          2 * egp * SQ:2 * (egp + 1) * SQ],
                in_=ot[:])

        # late V chunks: woven into phase B as PE filler for the
        # Act-latency-bound early attention batches
        for st in range(4, NST):
            deferred.append(
                ((1, st), lambda st=st:
                 v_chunk(1, st, pool=dfr_p, dve_copy=True)))

        # ---- phase B: attention (per s-chunk, both head pairs) ---------
        pending_fin = [None]

        for j in range(NJ):
            ndiag = 4 * j + 4
            for pair in range(2):
                drain_v(pair, 4 * j + 3)
                apsum = [acc_p.tile([128, 4, 128], f32, tag="acc",
                                    name="apsum") for _ in range(2)]
                dpsum = dps_p.tile([128, SQ], f32, tag="dps", name="dpsum")

                def emit_pv(i, r, pts, _ap=apsum, _dp=dpsum, _pair=pair,
                            _nd=ndiag):
                    first = (i == 0)
                    last = (i == _nd - 1)
                    s0 = r if r > 0 else 0
                    for hh in range(2):
                        for s in range(s0, 4):
                            nc.tensor.matmul(
                                _ap[hh][:, s, :],
                                pts[hh][:, ts(s, 128)],
                                vs[_pair][:, i, ts(hh, 128)],
                                start=(first and s == s0),
                                stop=(last and s == 3),
                                skip_group_check=True)
                            c = hh * 4 + s
                            nc.tensor.matmul(
                                _dp[:, c:c + 1],
                                pts[hh][:, ts(s, 128)],
                                ones_col[:],
                                start=(first and hh == 0 and s == s0),
                                stop=(last and hh == 1 and s == 3),
                                skip_group_check=True)

                prev = None   # (i, r, p_tiles)
                for i in range(ndiag):
                    r = i - 4 * j   # >=0 on diagonal tiles
                    off = 128 * r if r > 0 else 0
                    lg = []
                    for hh in range(2):
                        t = lg_p.tile([128, SQ], f32, tag="ps", name="lg")
                        nc.tensor.matmul(
                            t[:, off:SQ],
                            kT[pair][hh][:, ts(i, 128)],
                            qT[pair][hh][:, j * SQ + off:(j + 1) * SQ],
                            start=True, stop=(r < 0))
                        if r >= 0:
                            # causal mask folded into the psum group: adds
                            # the [-9e15] triangle via ident.T @ cmask
                            nc.tensor.matmul(
                                t[:, off:off + 128], ident[:], cmask[:],
                                start=False, stop=True,
                                skip_group_check=True)
                        lg.append(t)
                    pts = []
                    for hh in range(2):
                        p_t = p_p.tile([128, SQ], bf16, tag="p", name="p_t")
                        nc.scalar.activation(
                            p_t[:, off:SQ], lg[hh][:, off:SQ],
                            mybir.ActivationFunctionType.Exp,
                            scale=inv_sqrt_hd)
                        pts.append(p_t)
                    if i == 0 and pending_fin[0] is not None:
                        pending_fin[0]()
                        pending_fin[0] = None
                    if prev is not None:
                        emit_pv(*prev)
                    if i >= 1:
                        pop_deferred((2 if j < 3 else 3) if r >= 0 else 1)
                    prev = (i, r, pts)
                emit_pv(*prev)

                def fin(_ap=apsum, _dp=dpsum, _j=j, _pair=pair):
                    # 1/(WS * denom); normalize each [sq, hd] slice; queue
                    # the re-transposes (and o_proj once the chunk is done)
                    rc = rc_p.tile([128, 8], f32, tag="rc", name="rc")
                    nc.vector.reciprocal(rc[:], _dp[:, 0:8])
                    asq = [asq_p.tile([128, 4, 128], bf16, tag="asq",
                                      name="asq") for _ in range(2)]
                    for hh in range(2):
                        nc.vector.tensor_mul(
                            asq[hh][:, :, :], _ap[hh][:, :, :],
                            rc[:, 4 * hh:4 * hh + 4].unsqueeze(2)
                            .broadcast_to([128, 4, 128]))
                    for hh in range(2):
                        for s in range(4):
                            deferred.append(
                                (None, lambda hh=hh, s=s, asq=asq:
                                 transp_thunk(_j, _pair, hh, s, asq)))
                    if _pair == 1:
                        for st in range(4 * _j, 4 * _j + 4):
                            for egp in range(2):
                                deferred.append(
                                    (None, lambda st=st, egp=egp:
                                     oproj_thunk(st, egp)))
                pending_fin[0] = fin
        pending_fin[0]()
        pop_deferred(len(deferred))

    nc.compile()
    return nc


def _host_inputs(x, w_qkv, b_qkv, w_o):
    """Build the 8 per-core input maps."""
    x = np.asarray(x, dtype=np.float32)
    w_qkv = np.asarray(w_qkv, dtype=np.float32)
    b_qkv = np.asarray(b_qkv, dtype=np.float32)
    w_o = np.asarray(w_o, dtype=np.float32)
    use_bias = bool(np.any(b_qkv != 0.0))

    # rope tables (reference swaps sin/cos roles; we follow the math:
    # q_rot = q*sin(emb) + rotate_half(q)*cos(emb)).  Tables are divided
    # by WS to cancel the fp8 weight scaling.
    inv_freq = 1.0 / (10000.0 ** (np.arange(0, HD, 2, dtype=np.float32) / HD))
    t = np.arange(S, dtype=np.float32)
    freq = np.einsum("s,f->sf", t, inv_freq)          # [S, 64]
    sinT = (np.sin(freq).T / WS).astype(np.float32)   # [64, S]
    cosT = (np.cos(freq).T / WS).astype(np.float32)
    stab = np.concatenate([sinT, sinT], 0).astype(BF16)   # [128, S]
    ctab = np.concatenate([cosT, cosT], 0).astype(BF16)

    p_idx = np.arange(128)[:, None]
    f_idx = np.arange(128)[None, :]
    cmask = np.where(f_idx >= p_idx, 0.0, -9e15).astype(BF16)
    ident = np.eye(128, dtype=np.float32).astype(BF16)

    def head_w(h):
        base = h * 3 * HD
        return (w_qkv[:, base:base + HD],
                w_qkv[:, base + HD:base + 2 * HD],
                w_qkv[:, base + 2 * HD:base + 3 * HD])

    def head_b(h):
        base = h * 3 * HD
        return (b_qkv[base:base + HD],
                b_qkv[base + HD:base + 2 * HD],
                b_qkv[base + 2 * HD:base + 3 * HD])

    def fp8split(a):
        a8 = a.astype(F8)
        ar = (a - a8.astype(np.float32)).astype(F8)
        return a8, ar

    in_maps = []
    for c in range(N_CORES):
        b = c // 4
        heads = [4 * (c % 4) + i for i in range(4)]
        x8, xr = fp8split(np.ascontiguousarray(x[b].T))

        mats, bvec = [], []
        for pair in range(2):
            ha, hb = heads[2 * pair], heads[2 * pair + 1]
            wq_a, wk_a, wv_a = head_w(ha)
            wq_b, wk_b, wv_b = head_w(hb)
            bq_a, bk_a, bv_a = head_b(ha)
            bq_b, bk_b, bv_b = head_b(hb)
            mats += [
                np.concatenate([wq_a[:, :64], wq_b[:, :64]], 1),
                np.concatenate([wq_a[:, 64:], wq_b[:, 64:]], 1),
                np.concatenate([wk_a[:, :64], wk_b[:, :64]], 1),
                np.concatenate([wk_a[:, 64:], wk_b[:, 64:]], 1),
                wv_a, wv_b,
            ]
            bvec += [
                np.concatenate([bq_a[:64], bq_b[:64]]),
                np.concatenate([bq_a[64:], bq_b[64:]]),
                np.concatenate([bk_a[:64], bk_b[:64]]),
                np.concatenate([bk_a[64:], bk_b[64:]]),
                bv_a, bv_b,
            ]
        wq_all = np.concatenate(mats, 1) * WS                  # [D, 1536]
        w8, wr = fp8split(wq_all.astype(np.float32))
        wo_all = np.concatenate(
            [w_o[h * HD:(h + 1) * HD, :] for h in heads], 0) * WS
        # [4*128, D] -> [hp, p, ki, D] DoubleRow head-pair layout
        wo_hp = np.ascontiguousarray(
            wo_all.reshape(2, 2, HD, D).transpose(0, 2, 1, 3))
        wo8, wor = fp8split(wo_hp.astype(np.float32))

        im = {
            "x8": x8, "xr": xr, "w8": w8, "wr": wr,
            "wo8": wo8, "wor": wor,
            "stab": stab, "ctab": ctab, "cmask": cmask, "ident": ident,
        }
        if use_bias:
            im["bq"] = (np.concatenate(bvec)[None, :] * WS).astype(BF16)
        in_maps.append(im)
    return in_maps, use_bias


def _run(in_maps, use_bias, trace=False):
    from concourse.bass_utils import run_bass_kernel_spmd
    key = ("nc", use_bias)
    if key not in _MODULE_CACHE:
        _MODULE_CACHE[key] = _build_module(use_bias)
        _MODULE_CACHE["nc"] = _MODULE_CACHE[key]
    nc = _MODULE_CACHE[key]
    return nc, run_bass_kernel_spmd(nc, in_maps, core_ids=list(range(N_CORES)),
                                    trace=trace)


def kernel(x, w_qkv, b_qkv, w_o, b_o, _trace=False, _return_res=False):
    in_maps, use_bias = _host_inputs(x, w_qkv, b_qkv, w_o)
    nc, res = _run(in_maps, use_bias, trace=_trace)
    out = np.zeros((2, S, D), dtype=np.float32)
    for c in range(N_CORES):
        out[c // 4] += np.asarray(res.results[c]["out"], dtype=np.float32)
    out *= 1.0 / (WS * WS)   # undo the attn and w_o fp8 scaling
    out += np.asarray(b_o, dtype=np.float32)[None, None, :]
    if _return_res:
        return out, res
    return out
